# revision 11
# baseline (speedup 1.0000x reference)
"""Trainium2 Bass kernel for nn_Cls_Loss_42331197670001.

Reference computation (N=128 samples, C=345 classes, A=512 features):
    dataW[n,c,:] = W[c] - W[labels[n]]
    sigma2[n,c]  = Lambda * dataW[n,c] @ Sigma[labels[n]] @ dataW[n,c]^T
    dW_dMean[n,c]= dataW[n,c] . (mean_target-mean_source)[labels[n]]
    aug = y_s + 0.5*sigma2 + Lambda*dW_dMean ;  loss = mean softmax-CE(aug, labels)

Everything depends on the sample n only through its label l, so the heavy
quadratic form is computed once per *unique* label.  With the symmetrized
S_l = Sigma_l + Sigma_l^T:
    (W_c - W_l) Sigma_l (W_c - W_l)^T
        = 0.5 * d_S(l,c) - b(l,c) + 0.5 * s(l)
    d_S(l,c) = W_c S_l W_c^T          <- the only O(C*A*A) term, done on device
    b(l,c)   = W_c . (S_l W_l)        <- O(C*A) per label, host numpy
    s(l)     = W_l . (S_l W_l)        <- O(A) per label, host numpy

Device kernel (SPMD over 8 cores, unique labels sharded across cores):
per label j:  T = W @ S_j via 12 bf16 matmuls (PSUM [128,3,512], C padded
to 384 with zero rows so every matmul has M=128), then a fused DVE
tensor_tensor_reduce rowdot  d_S(c) = sum_a T[c,a]*W[c,a].
S symmetry means lhsT for the matmul is just W^T -- no transposes anywhere.
bf16 operands: 1 cycle/row on PE (fp32 would be 4) and half the HBM traffic;
the resulting |err| on the final scalar loss is ~1e-5 relative.
"""

import math
import sys

import numpy as np

try:
    import concourse.bass as bass
except ImportError:  # harness runs from a bare directory
    sys.path.insert(0, "/opt/trn_rl_repo")
    import concourse.bass as bass

import ml_dtypes

import concourse.mybir as mybir
import concourse.tile as tile
from concourse import bacc
from concourse.bass import ts
from concourse.bass_utils import run_bass_kernel_spmd

N_CORES = 8
A = 512          # feature dim
C = 345          # class count
C_PAD = 384      # 3 * 128
A_CHUNKS = A // 128   # 4
C_TILES = C_PAD // 128  # 3

BF16 = mybir.dt.bfloat16
F32 = mybir.dt.float32


def build_nc(u_pc: int, s_bufs: int = 4) -> bass.Bass:
    """Bass module: per core, u_pc labels; computes dout[p,t,j] = d_S(c=128t+p, j)."""
    nc = bacc.Bacc()
    wt = nc.dram_tensor("wt", [128, A_CHUNKS, C_PAD], BF16, kind="ExternalInput")
    wc = nc.dram_tensor("wc", [128, C_TILES, A], F32, kind="ExternalInput")
    sg = nc.dram_tensor("sg", [u_pc, 128, A_CHUNKS, A], BF16, kind="ExternalInput")
    dout = nc.dram_tensor("dout", [128, C_TILES, u_pc], F32, kind="ExternalOutput")

    with tile.TileContext(nc) as tc:
        with (
            tc.tile_pool(name="singles", bufs=1) as singles,
            tc.tile_pool(name="spool", bufs=s_bufs) as spool,
            tc.tile_pool(name="mpool", bufs=2) as mpool,
            tc.tile_pool(name="psum", bufs=2, space="PSUM") as ppool,
        ):
            wt_sb = singles.tile([128, A_CHUNKS, C_PAD], BF16)
            nc.sync.dma_start(out=wt_sb[:], in_=wt[:])
            wc_sb = singles.tile([128, C_TILES, A], F32)
            nc.sync.dma_start(out=wc_sb[:], in_=wc[:])
            d_sb = singles.tile([128, C_TILES, u_pc], F32)
            # The DVE TT encoding fits a single sync-wait. Absorb the wc DMA
            # wait into a throwaway DVE op so the first real tensor_tensor
            # only waits on the PE semaphore.
            scratch = singles.tile([128, 1], F32)
            nc.vector.tensor_copy(scratch[:], wc_sb[:, 0, 0:1])

            for j in range(u_pc):
                s_sb = spool.tile([128, A_CHUNKS, A], BF16, tag="s")
                nc.sync.dma_start(out=s_sb[:], in_=sg[j])
                ps = ppool.tile([128, C_TILES, A], F32, tag="ps")
                for t in range(C_TILES):
                    for k in range(A_CHUNKS):
                        nc.tensor.matmul(
                            ps[:, t, :],
                            lhsT=wt_sb[:, k, ts(t, 128)],
                            rhs=s_sb[:, k, :],
                            start=(k == 0),
                            stop=(k == A_CHUNKS - 1),
                        )
                m_sb = mpool.tile([128, C_TILES, A], F32, tag="m")
                for t in range(C_TILES):
                    nc.vector.tensor_tensor(
                        out=m_sb[:, t, :],
                        in0=ps[:, t, :],
                        in1=wc_sb[:, t, :],
                        op=mybir.AluOpType.mult,
                    )
                nc.vector.reduce_sum(
                    out=d_sb[:, :, j], in_=m_sb[:], axis=mybir.AxisListType.X
                )
            nc.sync.dma_start(out=dout[:], in_=d_sb[:])
    nc.compile()
    return nc


def host_pack(fc_weight: np.ndarray, lab_pad: np.ndarray, cov: np.ndarray):
    """Build the per-core device inputs. Returns (wt, wc, sg_all, S_f32)."""
    w_pad = np.zeros((C_PAD, A), np.float32)
    w_pad[:C] = fc_weight
    wt = np.ascontiguousarray(
        w_pad.T.reshape(A_CHUNKS, 128, C_PAD).transpose(1, 0, 2)
    ).astype(ml_dtypes.bfloat16)
    wc = np.ascontiguousarray(
        w_pad.reshape(C_TILES, 128, A).transpose(1, 0, 2)
    ).astype(np.float32)
    sgath = cov[lab_pad]                       # [U_pad, A, A]
    s_sym = sgath + sgath.transpose(0, 2, 1)   # Sigma + Sigma^T, float32
    sg_all = np.ascontiguousarray(
        s_sym.reshape(-1, A_CHUNKS, 128, A).transpose(0, 2, 1, 3)
    ).astype(ml_dtypes.bfloat16)
    return wt, wc, sg_all, s_sym


_NC_CACHE: dict[int, bass.Bass] = {}


def _device_dS(fc_weight, uniq, cov):
    """Run the Bass kernel on 8 cores; returns (d_S [U, C] float64, S_sym [U,A,A])."""
    U = len(uniq)
    u_pc = math.ceil(U / N_CORES)
    u_pad = u_pc * N_CORES
    lab_pad = np.concatenate([uniq, np.full(u_pad - U, uniq[0], dtype=uniq.dtype)])
    wt, wc, sg_all, s_sym = host_pack(fc_weight, lab_pad, cov)

    if u_pc not in _NC_CACHE:
        _NC_CACHE[u_pc] = build_nc(u_pc)
    nc = _NC_CACHE[u_pc]

    in_maps = [
        {"wt": wt, "wc": wc, "sg": np.ascontiguousarray(sg_all[i * u_pc : (i + 1) * u_pc])}
        for i in range(N_CORES)
    ]
    res = run_bass_kernel_spmd(nc, in_maps, core_ids=list(range(N_CORES)))
    parts = [r["dout"].transpose(2, 1, 0).reshape(u_pc, C_PAD) for r in res.results]
    d_s = np.concatenate(parts, axis=0)[:U, :C].astype(np.float64)
    return d_s, s_sym[:U]


def kernel(
    fc_weight,
    features_source,
    y_s,
    labels_source,
    Lambda,
    mean_source,
    mean_target,
    covariance_target,
):
    fc_weight = np.asarray(fc_weight, dtype=np.float32)
    y_s = np.asarray(y_s, dtype=np.float32)
    labels = np.asarray(labels_source).astype(np.int64)
    lam = float(np.asarray(Lambda))
    mean_source = np.asarray(mean_source, dtype=np.float32)
    mean_target = np.asarray(mean_target, dtype=np.float32)
    cov = np.asarray(covariance_target, dtype=np.float32)

    n = labels.shape[0]
    uniq, inv = np.unique(labels, return_inverse=True)

    d_s, s_sym = _device_dS(fc_weight, uniq, cov)

    # Cheap per-unique-label terms in float64 on host.
    w64 = fc_weight.astype(np.float64)
    wl = w64[uniq]                                         # [U, A]
    wv = np.einsum("uab,ub->ua", s_sym.astype(np.float64), wl)  # S_l @ W_l
    b = wv @ w64.T                                         # [U, C]
    s = np.einsum("ua,ua->u", wl, wv)                      # W_l S_l W_l^T
    quad = 0.5 * d_s - b + 0.5 * s[:, None]                # [U, C]

    d_mean = (mean_target - mean_source).astype(np.float64)[uniq]  # [U, A]
    g = d_mean @ w64.T                                     # [U, C]
    g_self = np.einsum("ua,ua->u", wl, d_mean)             # [U]

    aug = (
        y_s.astype(np.float64)
        + 0.5 * lam * quad[inv]
        + lam * (g[inv] - g_self[inv][:, None])
    )
    mx = aug.max(axis=1, keepdims=True)
    lse = mx[:, 0] + np.log(np.exp(aug - mx).sum(axis=1))
    nll = lse - aug[np.arange(n), labels]
    return np.array(nll.mean(), dtype=np.float32)


# revision 13
# speedup vs baseline: 1.2422x; 1.2422x over previous
"""Trainium2 Bass kernel for nn_Cls_Loss_42331197670001.

Reference computation (N=128 samples, C=345 classes, A=512 features):
    dataW[n,c,:] = W[c] - W[labels[n]]
    sigma2[n,c]  = Lambda * dataW[n,c] @ Sigma[labels[n]] @ dataW[n,c]^T
    dW_dMean[n,c]= dataW[n,c] . (mean_target-mean_source)[labels[n]]
    aug = y_s + 0.5*sigma2 + Lambda*dW_dMean ;  loss = mean softmax-CE(aug, labels)

Everything depends on the sample n only through its label l, so the heavy
quadratic form is computed once per *unique* label.  With the symmetrized
S_l = Sigma_l + Sigma_l^T:
    (W_c - W_l) Sigma_l (W_c - W_l)^T
        = 0.5 * d_S(l,c) - b(l,c) + 0.5 * s(l)
    d_S(l,c) = W_c S_l W_c^T          <- the only O(C*A*A) term, done on device
    b(l,c)   = W_c . (S_l W_l)        <- O(C*A) per label, host numpy
    s(l)     = W_l . (S_l W_l)        <- O(A) per label, host numpy

Device kernel (SPMD over 8 cores, unique labels sharded across cores):
per label j:  T = W @ S_j via 12 bf16 matmuls (PSUM [128,3,512], C padded
to 384 with zero rows so every matmul has M=128), then a fused DVE
tensor_tensor_reduce rowdot  d_S(c) = sum_a T[c,a]*W[c,a].
S symmetry means lhsT for the matmul is just W^T -- no transposes anywhere.
bf16 operands: 1 cycle/row on PE (fp32 would be 4) and half the HBM traffic;
the resulting |err| on the final scalar loss is ~1e-5 relative.
"""

import math
import sys

import numpy as np

try:
    import concourse.bass as bass
except ImportError:  # harness runs from a bare directory
    sys.path.insert(0, "/opt/trn_rl_repo")
    import concourse.bass as bass

import ml_dtypes

import concourse.mybir as mybir
import concourse.tile as tile
from concourse import bacc
from concourse.bass import ts
from concourse.bass_utils import run_bass_kernel_spmd

N_CORES = 8
A = 512          # feature dim
C = 345          # class count
C_PAD = 384      # 3 * 128
A_CHUNKS = A // 128   # 4
C_TILES = C_PAD // 128  # 3

BF16 = mybir.dt.bfloat16
F32 = mybir.dt.float32


def build_nc(u_pc: int, s_bufs: int = 4) -> bass.Bass:
    """Bass module: per core, u_pc labels; computes dout[p,t,j] = d_S(c=128t+p, j)."""
    nc = bacc.Bacc()
    wt = nc.dram_tensor("wt", [128, A_CHUNKS, C_PAD], BF16, kind="ExternalInput")
    wc = nc.dram_tensor("wc", [128, C_TILES, A], F32, kind="ExternalInput")
    sg = nc.dram_tensor("sg", [u_pc, 128, A_CHUNKS, A], BF16, kind="ExternalInput")
    dout = nc.dram_tensor("dout", [128, C_TILES, u_pc], F32, kind="ExternalOutput")

    with tile.TileContext(nc) as tc:
        with (
            tc.tile_pool(name="singles", bufs=1) as singles,
            tc.tile_pool(name="spool", bufs=s_bufs) as spool,
            tc.tile_pool(name="mpool", bufs=3) as mpool,
            tc.tile_pool(name="psum", bufs=8, space="PSUM") as ppool,
        ):
            wt_sb = singles.tile([128, A_CHUNKS, C_PAD], BF16)
            nc.sync.dma_start(out=wt_sb[:], in_=wt[:])
            wc_sb = singles.tile([128, C_TILES, A], F32)
            nc.sync.dma_start(out=wc_sb[:], in_=wc[:])
            d_sb = singles.tile([128, C_TILES, u_pc], F32)
            # The DVE TT encoding fits a single sync-wait. Absorb the wc DMA
            # wait into a throwaway DVE op so the first real tensor_tensor
            # only waits on the PE semaphore.
            scratch = singles.tile([128, 1], F32)
            nc.vector.tensor_copy(scratch[:], wc_sb[:, 0, 0:1])

            for j in range(u_pc):
                s_sb = spool.tile([128, A_CHUNKS, A], BF16, tag="s")
                nc.sync.dma_start(out=s_sb[:], in_=sg[j])
                for t in range(C_TILES):
                    ps = ppool.tile([128, A], F32, tag="ps")
                    for k in range(A_CHUNKS):
                        nc.tensor.matmul(
                            ps[:, :],
                            lhsT=wt_sb[:, k, ts(t, 128)],
                            rhs=s_sb[:, k, :],
                            start=(k == 0),
                            stop=(k == A_CHUNKS - 1),
                        )
                    m_sb = mpool.tile([128, A], F32, tag="m")
                    nc.vector.tensor_tensor(
                        out=m_sb[:],
                        in0=ps[:],
                        in1=wc_sb[:, t, :],
                        op=mybir.AluOpType.mult,
                    )
                    # Free-dim sum on the otherwise-idle ScalarE so the DVE
                    # only pays the PSUM-read multiply.
                    act_o = mpool.tile([128, A], BF16, tag="act_o")
                    nc.scalar.activation(
                        out=act_o[:],
                        in_=m_sb[:],
                        func=mybir.ActivationFunctionType.Copy,
                        accum_out=d_sb[:, t, j : j + 1],
                    )
            nc.sync.dma_start(out=dout[:], in_=d_sb[:])
    nc.compile()
    return nc


def host_pack(fc_weight: np.ndarray, lab_pad: np.ndarray, cov: np.ndarray):
    """Build the per-core device inputs. Returns (wt, wc, sg_all, S_f32)."""
    w_pad = np.zeros((C_PAD, A), np.float32)
    w_pad[:C] = fc_weight
    wt = np.ascontiguousarray(
        w_pad.T.reshape(A_CHUNKS, 128, C_PAD).transpose(1, 0, 2)
    ).astype(ml_dtypes.bfloat16)
    wc = np.ascontiguousarray(
        w_pad.reshape(C_TILES, 128, A).transpose(1, 0, 2)
    ).astype(np.float32)
    sgath = cov[lab_pad]                       # [U_pad, A, A]
    s_sym = sgath + sgath.transpose(0, 2, 1)   # Sigma + Sigma^T, float32
    sg_all = np.ascontiguousarray(
        s_sym.reshape(-1, A_CHUNKS, 128, A).transpose(0, 2, 1, 3)
    ).astype(ml_dtypes.bfloat16)
    return wt, wc, sg_all, s_sym


_NC_CACHE: dict[int, bass.Bass] = {}


def _device_dS(fc_weight, uniq, cov):
    """Run the Bass kernel on 8 cores; returns (d_S [U, C] float64, S_sym [U,A,A])."""
    U = len(uniq)
    u_pc = math.ceil(U / N_CORES)
    u_pad = u_pc * N_CORES
    lab_pad = np.concatenate([uniq, np.full(u_pad - U, uniq[0], dtype=uniq.dtype)])
    wt, wc, sg_all, s_sym = host_pack(fc_weight, lab_pad, cov)

    if u_pc not in _NC_CACHE:
        _NC_CACHE[u_pc] = build_nc(u_pc)
    nc = _NC_CACHE[u_pc]

    in_maps = [
        {"wt": wt, "wc": wc, "sg": np.ascontiguousarray(sg_all[i * u_pc : (i + 1) * u_pc])}
        for i in range(N_CORES)
    ]
    res = run_bass_kernel_spmd(nc, in_maps, core_ids=list(range(N_CORES)))
    parts = [r["dout"].transpose(2, 1, 0).reshape(u_pc, C_PAD) for r in res.results]
    d_s = np.concatenate(parts, axis=0)[:U, :C].astype(np.float64)
    return d_s, s_sym[:U]


def kernel(
    fc_weight,
    features_source,
    y_s,
    labels_source,
    Lambda,
    mean_source,
    mean_target,
    covariance_target,
):
    fc_weight = np.asarray(fc_weight, dtype=np.float32)
    y_s = np.asarray(y_s, dtype=np.float32)
    labels = np.asarray(labels_source).astype(np.int64)
    lam = float(np.asarray(Lambda))
    mean_source = np.asarray(mean_source, dtype=np.float32)
    mean_target = np.asarray(mean_target, dtype=np.float32)
    cov = np.asarray(covariance_target, dtype=np.float32)

    n = labels.shape[0]
    uniq, inv = np.unique(labels, return_inverse=True)

    d_s, s_sym = _device_dS(fc_weight, uniq, cov)

    # Cheap per-unique-label terms in float64 on host.
    w64 = fc_weight.astype(np.float64)
    wl = w64[uniq]                                         # [U, A]
    wv = np.einsum("uab,ub->ua", s_sym.astype(np.float64), wl)  # S_l @ W_l
    b = wv @ w64.T                                         # [U, C]
    s = np.einsum("ua,ua->u", wl, wv)                      # W_l S_l W_l^T
    quad = 0.5 * d_s - b + 0.5 * s[:, None]                # [U, C]

    d_mean = (mean_target - mean_source).astype(np.float64)[uniq]  # [U, A]
    g = d_mean @ w64.T                                     # [U, C]
    g_self = np.einsum("ua,ua->u", wl, d_mean)             # [U]

    aug = (
        y_s.astype(np.float64)
        + 0.5 * lam * quad[inv]
        + lam * (g[inv] - g_self[inv][:, None])
    )
    mx = aug.max(axis=1, keepdims=True)
    lse = mx[:, 0] + np.log(np.exp(aug - mx).sum(axis=1))
    nll = lse - aug[np.arange(n), labels]
    return np.array(nll.mean(), dtype=np.float32)


# revision 14
# speedup vs baseline: 1.3321x; 1.0724x over previous
"""Trainium2 Bass kernel for nn_Cls_Loss_42331197670001.

Reference computation (N=128 samples, C=345 classes, A=512 features):
    dataW[n,c,:] = W[c] - W[labels[n]]
    sigma2[n,c]  = Lambda * dataW[n,c] @ Sigma[labels[n]] @ dataW[n,c]^T
    dW_dMean[n,c]= dataW[n,c] . (mean_target-mean_source)[labels[n]]
    aug = y_s + 0.5*sigma2 + Lambda*dW_dMean ;  loss = mean softmax-CE(aug, labels)

Everything depends on the sample n only through its label l, so the heavy
quadratic form is computed once per *unique* label.  With the symmetrized
S_l = Sigma_l + Sigma_l^T:
    (W_c - W_l) Sigma_l (W_c - W_l)^T = 0.5*d_S(l,c) - b(l,c) + 0.5*s(l)
    d_S(l,c) = W_c S_l W_c^T          <- the only O(C*A*A) term, on device
    b, s, mean-shift, softmax-CE      <- tiny, host numpy in float64

Device kernel (SPMD over 8 cores, unique labels sharded across cores),
per label j, all in the TRANSPOSED layout (S symmetric => S @ W^T needs no
transposes):
    Tt = (S*32) @ (W^T*16)      fp8e4 DoubleRow matmuls, PSUM [b, c]
    m  = Tt (*) W^T_fp32        DVE tensor_tensor, PSUM-read, bf16 out
    d  = ones^T @ m             partition-dim sum = 4 tiny PE matmuls
    d_S = d / 512 on host.
fp8 inputs halve DMA and double PE throughput; the scale factors (powers
of two) keep values in e4m3's sweet range. Resulting error on the final
scalar loss is ~1e-5 relative.
"""

import math
import sys

import numpy as np

try:
    import concourse.bass as bass
except ImportError:  # harness runs from a bare directory
    sys.path.insert(0, "/opt/trn_rl_repo")
    import concourse.bass as bass

import ml_dtypes

import concourse.mybir as mybir
import concourse.tile as tile
from concourse import bacc
from concourse.bass import ts
from concourse.bass_utils import run_bass_kernel_spmd

N_CORES = 8
A = 512          # feature dim
C = 345          # class count
C_PAD = 384      # 3 * 128
A_CHUNKS = A // 128   # 4

W_SCALE = 16.0
S_SCALE = 32.0
OUT_SCALE = W_SCALE * S_SCALE

FP8 = mybir.dt.float8e4
BF16 = mybir.dt.bfloat16
F32 = mybir.dt.float32
FP8_NP = ml_dtypes.float8_e4m3


def build_nc(u_pc: int, s_bufs: int = 6) -> bass.Bass:
    """Per core: u_pc labels; dout[j, c] = (W S_j W^T)[c,c] * OUT_SCALE."""
    nc = bacc.Bacc()
    wt8 = nc.dram_tensor("wt8", [128, A_CHUNKS, C_PAD], FP8, kind="ExternalInput")
    wt32 = nc.dram_tensor("wt32", [128, A_CHUNKS, C_PAD], F32, kind="ExternalInput")
    sg = nc.dram_tensor("sg", [u_pc, 128, A_CHUNKS, A], FP8, kind="ExternalInput")
    dout = nc.dram_tensor("dout", [u_pc, C_PAD], F32, kind="ExternalOutput")

    with tile.TileContext(nc) as tc:
        with (
            tc.tile_pool(name="singles", bufs=1) as singles,
            tc.tile_pool(name="spool", bufs=s_bufs) as spool,
            tc.tile_pool(name="mpool", bufs=3) as mpool,
            tc.tile_pool(name="psum", bufs=3, space="PSUM") as ppool,
            tc.tile_pool(name="cspsum", bufs=2, space="PSUM") as cspool,
        ):
            wt8_sb = singles.tile([128, A_CHUNKS, C_PAD], FP8)
            nc.sync.dma_start(out=wt8_sb[:], in_=wt8[:])
            wt32_sb = singles.tile([128, A_CHUNKS, C_PAD], F32)
            nc.sync.dma_start(out=wt32_sb[:], in_=wt32[:])
            ones_sb = singles.tile([128, 1], BF16)
            nc.vector.memset(ones_sb[:], 1.0)
            d_sb = singles.tile([1, u_pc, C_PAD], F32)
            # DVE/PE instruction encodings fit one sync-wait; absorb the
            # weight-DMA waits into throwaway DVE ops up front.
            scr = singles.tile([128, 1], F32)
            nc.vector.tensor_copy(scr[:], wt32_sb[:, 0, 0:1])
            scr8 = singles.tile([128, 1], BF16)
            nc.vector.tensor_copy(scr8[:], wt8_sb[:, 0, 0:1])

            for j in range(u_pc):
                s_sb = spool.tile([128, A_CHUNKS, A], FP8, tag="s")
                nc.sync.dma_start(out=s_sb[:], in_=sg[j])
                m_sb = mpool.tile([128, A_CHUNKS, C_PAD], BF16, tag="m")
                # Tt[b, c] = sum_a S[a, b] * Wt[a, c], b-tiles pairs in PSUM.
                for p in range(2):  # b-tile pairs (0,1) and (2,3)
                    ps = ppool.tile([128, 2, A], F32, tag="ps")
                    for k in (0, 2):  # a-chunk pairs, DoubleRow
                        for i in range(2):
                            nc.tensor.matmul(
                                ps[:, i, :C_PAD],
                                lhsT=s_sb[:, k : k + 2, ts(2 * p + i, 128)],
                                rhs=wt8_sb[:, k : k + 2, :],
                                start=(k == 0),
                                stop=(k == 2),
                                perf_mode=mybir.MatmulPerfMode.DoubleRow,
                            )
                    nc.vector.tensor_tensor(
                        out=m_sb[:, 2 * p : 2 * p + 2, :],
                        in0=ps[:, :, :C_PAD],
                        in1=wt32_sb[:, 2 * p : 2 * p + 2, :],
                        op=mybir.AluOpType.mult,
                    )
                # d[c] = sum_b m[b, c]: partition reduce via ones-matmul.
                cs = cspool.tile([1, A], F32, tag="cs")
                for k in range(A_CHUNKS):
                    nc.tensor.matmul(
                        cs[:, :C_PAD],
                        lhsT=ones_sb[:],
                        rhs=m_sb[:, k, :],
                        start=(k == 0),
                        stop=(k == A_CHUNKS - 1),
                    )
                nc.scalar.copy(out=d_sb[:, j, :], in_=cs[:, :C_PAD])
            nc.sync.dma_start(out=dout[:], in_=d_sb[0])
    nc.compile()
    return nc


def host_pack(fc_weight: np.ndarray, lab_pad: np.ndarray, cov: np.ndarray):
    """Build device inputs. Returns (wt8, wt32, sg_all, S_f32)."""
    w_pad = np.zeros((C_PAD, A), np.float32)
    w_pad[:C] = fc_weight
    wt = np.ascontiguousarray(
        w_pad.T.reshape(A_CHUNKS, 128, C_PAD).transpose(1, 0, 2)
    )
    wt8 = (wt * W_SCALE).astype(FP8_NP)
    sgath = cov[lab_pad]                       # [U_pad, A, A]
    s_sym = sgath + sgath.transpose(0, 2, 1)   # Sigma + Sigma^T, float32
    sg_all = (
        np.ascontiguousarray(
            s_sym.reshape(-1, A_CHUNKS, 128, A).transpose(0, 2, 1, 3)
        )
        * S_SCALE
    ).astype(FP8_NP)
    return wt8, wt, sg_all, s_sym


_NC_CACHE: dict[int, bass.Bass] = {}


def _device_dS(fc_weight, uniq, cov):
    """Run the Bass kernel on 8 cores; returns (d_S [U, C] float64, S_sym [U,A,A])."""
    U = len(uniq)
    u_pc = math.ceil(U / N_CORES)
    u_pad = u_pc * N_CORES
    lab_pad = np.concatenate([uniq, np.full(u_pad - U, uniq[0], dtype=uniq.dtype)])
    wt8, wt32, sg_all, s_sym = host_pack(fc_weight, lab_pad, cov)

    if u_pc not in _NC_CACHE:
        _NC_CACHE[u_pc] = build_nc(u_pc)
    nc = _NC_CACHE[u_pc]

    in_maps = [
        {
            "wt8": wt8,
            "wt32": wt32,
            "sg": np.ascontiguousarray(sg_all[i * u_pc : (i + 1) * u_pc]),
        }
        for i in range(N_CORES)
    ]
    res = run_bass_kernel_spmd(nc, in_maps, core_ids=list(range(N_CORES)))
    d_s = np.concatenate([r["dout"] for r in res.results], axis=0)[:U, :C]
    return d_s.astype(np.float64) / OUT_SCALE, s_sym[:U]


def kernel(
    fc_weight,
    features_source,
    y_s,
    labels_source,
    Lambda,
    mean_source,
    mean_target,
    covariance_target,
):
    fc_weight = np.asarray(fc_weight, dtype=np.float32)
    y_s = np.asarray(y_s, dtype=np.float32)
    labels = np.asarray(labels_source).astype(np.int64)
    lam = float(np.asarray(Lambda))
    mean_source = np.asarray(mean_source, dtype=np.float32)
    mean_target = np.asarray(mean_target, dtype=np.float32)
    cov = np.asarray(covariance_target, dtype=np.float32)

    n = labels.shape[0]
    uniq, inv = np.unique(labels, return_inverse=True)

    d_s, s_sym = _device_dS(fc_weight, uniq, cov)

    # Cheap per-unique-label terms in float64 on host.
    w64 = fc_weight.astype(np.float64)
    wl = w64[uniq]                                         # [U, A]
    wv = np.einsum("uab,ub->ua", s_sym.astype(np.float64), wl)  # S_l @ W_l
    b = wv @ w64.T                                         # [U, C]
    s = np.einsum("ua,ua->u", wl, wv)                      # W_l S_l W_l^T
    quad = 0.5 * d_s - b + 0.5 * s[:, None]                # [U, C]

    d_mean = (mean_target - mean_source).astype(np.float64)[uniq]  # [U, A]
    g = d_mean @ w64.T                                     # [U, C]
    g_self = np.einsum("ua,ua->u", wl, d_mean)             # [U]

    aug = (
        y_s.astype(np.float64)
        + 0.5 * lam * quad[inv]
        + lam * (g[inv] - g_self[inv][:, None])
    )
    mx = aug.max(axis=1, keepdims=True)
    lse = mx[:, 0] + np.log(np.exp(aug - mx).sum(axis=1))
    nll = lse - aug[np.arange(n), labels]
    return np.array(nll.mean(), dtype=np.float32)
